# revision 1
# baseline (speedup 1.0000x reference)
"""CrossAttention Trainium2 kernel.

Problem (hardcoded): B=8, T=256, S=4096, E=512, KV=768, H=8, D=64.
Sharding: data-parallel over B — one batch per NeuronCore (8 cores).

Per-core dataflow (one batch, all layouts staged host-side):
  inputs (bf16 unless noted):
    ctxT  [768, 4096]   = context[b].T          (KV on partitions)
    xT    [512, 256]    = x[b].T
    m01   [128, 32] f32 = 1.0 where key kept, 0.0 where masked (s=sc*128+p)
    wqT   [512, 512]    = Wq.T * D^-0.5  (scale folded, exact pow2)
    wkvT  [768, 1024]   = Wkv.T
    woT   [512, 512]    = Wo.T
    bo_r  [128, 4] f32  = bo.reshape(4,128).T
  device:
    QT    = wqT.T @ xT            -> [512c, 256t]   (c-major, 4 chunks)
    KT    = wkvT[:, :512].T @ ctxT -> [512c, 4096s] (c-major, 4 chunks = head pairs)
    V'    = ctxT.T @ wkvT[:, 512:] -> [4096s, 8h*65] (64 vals + ones col per head),
            rows multiplied by m01 (mask folded into V' => no -inf anywhere)
    scoresT[s,t] per head = KT_h slices as lhsT, QT_h as rhs (K=64, head pair
            packed into PE row groups 0:64 / 64:128)
    expsT = Exp(scoresT)  (no max subtraction needed: |scores| <~ 8)
    PV    = V'_h-as-lhsT @ expsT -> [65, 256] psum; row 64 = softmax denom
    norm  = reciprocal(denom) broadcast via K=1 fp32 matmul; OT = PV * recip
    outT  = woT.T @ OT + bo -> [512e, 256t] -> host transposes back.

ctx DMA is quartered along S and kv-proj consumption follows arrival order.
Scores for 4 s-chunks of one head land in one [128,1024] psum tile so a
single ACTIVATE(Exp) covers them (ACT op overhead would otherwise bind).
"""

import sys

sys.path.insert(0, "/opt/trn_rl_repo")

import numpy as np
import ml_dtypes
from contextlib import ExitStack

import concourse.bass as bass
import concourse.bacc as bacc
import concourse.tile as tile
from concourse import mybir
from concourse import bass_utils

BF16 = mybir.dt.bfloat16
F32 = mybir.dt.float32
NPBF16 = ml_dtypes.bfloat16

B, T, S, E, KV, H, D = 8, 256, 4096, 512, 768, 8, 64
NC_CORES = 8


def _build_program():
    nc = bacc.Bacc("TRN2", target_bir_lowering=False, debug=False)

    ctxT_d = nc.dram_tensor("ctxT", [KV, S], BF16, kind="ExternalInput").ap()
    xT_d = nc.dram_tensor("xT", [E, T], BF16, kind="ExternalInput").ap()
    m01_d = nc.dram_tensor("m01", [128, 32], F32, kind="ExternalInput").ap()
    wqT_d = nc.dram_tensor("wqT", [E, 512], BF16, kind="ExternalInput").ap()
    wkvT_d = nc.dram_tensor("wkvT", [KV, 1024], BF16, kind="ExternalInput").ap()
    woT_d = nc.dram_tensor("woT", [512, E], BF16, kind="ExternalInput").ap()
    bo_d = nc.dram_tensor("bo_r", [128, 4], F32, kind="ExternalInput").ap()
    outT_d = nc.dram_tensor("outT", [4, 128, T], F32, kind="ExternalOutput").ap()

    ctxT_v = ctxT_d.rearrange("(c p) s -> c p s", p=128)  # [6,128,4096]
    xT_v = xT_d.rearrange("(c p) t -> c p t", p=128)  # [4,128,256]
    wqT_v = wqT_d.rearrange("(c p) m -> c p m", p=128)  # [4,128,512]
    wkvT_v = wkvT_d.rearrange("(c p) m -> c p m", p=128)  # [6,128,1024]
    woT_v = woT_d.rearrange("(c p) m -> c p m", p=128)  # [4,128,512]

    with tile.TileContext(nc) as tc, ExitStack() as ctx:
        const = ctx.enter_context(tc.tile_pool(name="const", bufs=1))
        work = ctx.enter_context(tc.tile_pool(name="work", bufs=2))
        p_pe = ctx.enter_context(tc.tile_pool(name="p_pe", bufs=3, space="PSUM"))
        p_pv = ctx.enter_context(tc.tile_pool(name="p_pv", bufs=2, space="PSUM"))

        # ---- static SBUF tensors -------------------------------------------
        # ctx quarters: ctx_t[c][q] = [128, 1024]
        ctx_t = [
            [
                const.tile([128, 1024], BF16, tag=f"ctx{c}_{q}", name=f"ctx{c}_{q}")
                for q in range(4)
            ]
            for c in range(6)
        ]
        kt_t = [
            const.tile([128, S], BF16, tag=f"kt{kc}", name=f"kt{kc}") for kc in range(4)
        ]
        vp_t = [
            const.tile([128, 8 * 65], BF16, tag=f"vp{sc}", name=f"vp{sc}")
            for sc in range(32)
        ]
        qt_t = [
            const.tile([128, T], BF16, tag=f"qt{qc}", name=f"qt{qc}") for qc in range(4)
        ]
        ot_t = [
            const.tile([128, T], BF16, tag=f"ot{cc}", name=f"ot{cc}") for cc in range(4)
        ]
        wq_t = [
            const.tile([128, 512], BF16, tag=f"wq{ec}", name=f"wq{ec}")
            for ec in range(4)
        ]
        wkv_t = [
            const.tile([128, 1024], BF16, tag=f"wkv{c}", name=f"wkv{c}")
            for c in range(6)
        ]
        wo_t = [
            const.tile([128, 512], BF16, tag=f"wo{cc}", name=f"wo{cc}")
            for cc in range(4)
        ]
        x_t = [
            const.tile([128, T], BF16, tag=f"x{ec}", name=f"x{ec}") for ec in range(4)
        ]
        pvacc_t = [
            const.tile([65, T], F32, tag=f"pvacc{h}", name=f"pvacc{h}") for h in range(8)
        ]
        den8_t = const.tile([8, T], F32, tag="den8")
        rec8_t = const.tile([8, T], F32, tag="rec8")
        rech_t = const.tile([1, 8 * T], F32, tag="rech")
        m01_t = const.tile([128, 32], F32, tag="m01")
        bo_t = const.tile([128, 4], F32, tag="bo")
        ones8_t = const.tile([128, 8], BF16, tag="ones8")
        ones64_t = const.tile([1, 64], F32, tag="ones64")

        # ---- loads ----------------------------------------------------------
        nc.vector.memset(ones8_t[:], 1.0)
        nc.vector.memset(ones64_t[:], 1.0)
        for ec in range(4):
            nc.gpsimd.dma_start(x_t[ec][:], xT_v[ec])
            nc.gpsimd.dma_start(wq_t[ec][:], wqT_v[ec])
        for c in range(6):
            nc.gpsimd.dma_start(wkv_t[c][:], wkvT_v[c])
        for c in range(3):  # first ctx quarter split across both DMA queues
            nc.sync.dma_start(ctx_t[c][0][:], ctxT_v[c][:, 0:1024])
        for c in range(3, 6):
            nc.gpsimd.dma_start(ctx_t[c][0][:], ctxT_v[c][:, 0:1024])
        for q in range(1, 4):
            for c in range(6):
                nc.sync.dma_start(
                    ctx_t[c][q][:], ctxT_v[c][:, q * 1024 : (q + 1) * 1024]
                )
        nc.gpsimd.dma_start(m01_t[:], m01_d)
        for cc in range(4):
            nc.gpsimd.dma_start(wo_t[cc][:], woT_v[cc])
        nc.gpsimd.dma_start(bo_t[:], bo_d)

        def ctx_slice(c, s0, n):
            q = s0 // 1024
            off = s0 - q * 1024
            return ctx_t[c][q][:, off : off + n]

        # ---- Q projection ---------------------------------------------------
        for qc in range(4):
            ps = p_pe.tile([128, 1024], F32, tag="pe")
            for ec in range(4):
                nc.tensor.matmul(
                    ps[:, 0:T],
                    lhsT=wq_t[ec][:, qc * 128 : (qc + 1) * 128],
                    rhs=x_t[ec][:],
                    start=(ec == 0),
                    stop=(ec == 3),
                )
            nc.vector.tensor_copy(qt_t[qc][:], ps[:, 0:T])

        # ---- interleaved KV projection + attention, per ctx quarter ---------
        # Attention group (kc, g) only needs ctx quarter g//2, so scores/exp/PV
        # for s-chunks of quarter q run right after that quarter's K/V proj.
        # PV accumulates per-quarter in PSUM, then adds into SBUF pvacc (DVE),
        # keeping only 2 PV psum banks live and the ACT exp work overlapped
        # with the next quarter's kv-proj matmuls.
        for q in range(4):
            for kc in range(4):
                ps = p_pe.tile([128, 1024], F32, tag="pe")
                for c in range(6):
                    for halfi in range(2):
                        nc.tensor.matmul(
                            ps[:, halfi * 512 : (halfi + 1) * 512],
                            lhsT=wkv_t[c][:, kc * 128 : (kc + 1) * 128],
                            rhs=ctx_slice(c, q * 1024 + halfi * 512, 512),
                            start=(c == 0),
                            stop=(c == 5),
                        )
                nc.vector.tensor_copy(
                    kt_t[kc][:, q * 1024 : (q + 1) * 1024], ps[:]
                )
            for sc in range(q * 8, (q + 1) * 8):
                ps = p_pe.tile([128, 1024], F32, tag="pe")
                for c in range(6):
                    nc.tensor.matmul(
                        ps[:, 0:512],
                        lhsT=ctx_slice(c, sc * 128, 128),
                        rhs=wkv_t[c][:, 512:1024],
                        start=(c == 0),
                        stop=(c == 5),
                    )
                dst = vp_t[sc][:].rearrange("p (h e) -> p h e", e=65)
                nc.vector.tensor_scalar_mul(
                    dst[:, :, 0:64],
                    ps[:, 0:512].rearrange("p (h d) -> p h d", d=64),
                    m01_t[:, sc : sc + 1],
                )
                nc.vector.tensor_scalar_mul(
                    dst[:, :, 64:65],
                    ones8_t[:].rearrange("p (h o) -> p h o", o=1),
                    m01_t[:, sc : sc + 1],
                )
            for kc in range(4):
                pvq0 = p_pv.tile([65, T], F32, tag="pv")
                pvq1 = p_pv.tile([65, T], F32, tag="pv")
                for g in (2 * q, 2 * q + 1):
                    pe0 = p_pe.tile([128, 1024], F32, tag="pe")
                    pe1 = p_pe.tile([128, 1024], F32, tag="pe")
                    for j in range(4):
                        sc = g * 4 + j
                        nc.tensor.matmul(
                            pe0[:, j * 256 : (j + 1) * 256],
                            lhsT=kt_t[kc][0:64, sc * 128 : (sc + 1) * 128],
                            rhs=qt_t[kc][0:64, :],
                            start=True,
                            stop=True,
                        )
                        nc.tensor.matmul(
                            pe1[:, j * 256 : (j + 1) * 256],
                            lhsT=kt_t[kc][64:128, sc * 128 : (sc + 1) * 128],
                            rhs=qt_t[kc][64:128, :],
                            start=True,
                            stop=True,
                        )
                    e0 = work.tile([128, 1024], BF16, tag="exp", bufs=6)
                    nc.scalar.activation(
                        e0[:], pe0[:], mybir.ActivationFunctionType.Exp
                    )
                    e1 = work.tile([128, 1024], BF16, tag="exp", bufs=6)
                    nc.scalar.activation(
                        e1[:], pe1[:], mybir.ActivationFunctionType.Exp
                    )
                    for j in range(4):
                        sc = g * 4 + j
                        nc.tensor.matmul(
                            pvq0[:],
                            lhsT=vp_t[sc][:, (2 * kc) * 65 : (2 * kc) * 65 + 65],
                            rhs=e0[:, j * 256 : (j + 1) * 256],
                            start=(g == 2 * q and j == 0),
                            stop=(g == 2 * q + 1 and j == 3),
                        )
                        nc.tensor.matmul(
                            pvq1[:],
                            lhsT=vp_t[sc][
                                :, (2 * kc + 1) * 65 : (2 * kc + 1) * 65 + 65
                            ],
                            rhs=e1[:, j * 256 : (j + 1) * 256],
                            start=(g == 2 * q and j == 0),
                            stop=(g == 2 * q + 1 and j == 3),
                        )
                if q == 0:
                    nc.vector.tensor_copy(pvacc_t[2 * kc][:], pvq0[:])
                    nc.vector.tensor_copy(pvacc_t[2 * kc + 1][:], pvq1[:])
                else:
                    nc.vector.tensor_add(
                        pvacc_t[2 * kc][:], pvacc_t[2 * kc][:], pvq0[:]
                    )
                    nc.vector.tensor_add(
                        pvacc_t[2 * kc + 1][:], pvacc_t[2 * kc + 1][:], pvq1[:]
                    )
                if q == 3:
                    nc.sync.dma_start(
                        den8_t[2 * kc : 2 * kc + 1, :], pvacc_t[2 * kc][64:65, :]
                    )
                    nc.gpsimd.dma_start(
                        den8_t[2 * kc + 1 : 2 * kc + 2, :],
                        pvacc_t[2 * kc + 1][64:65, :],
                    )

        # ---- deferred softmax normalization (off the PE critical path) ------
        nc.vector.reciprocal(rec8_t[:], den8_t[:])
        nc.sync.dma_start(
            rech_t[0:1, :].rearrange("p (h t) -> p h t", t=T), rec8_t[:, :]
        )
        for kc in range(4):
            bc0 = p_pv.tile([64, T], F32, tag="pv")
            nc.tensor.matmul(
                bc0[:],
                lhsT=ones64_t[:],
                rhs=rech_t[0:1, (2 * kc) * T : (2 * kc + 1) * T],
                start=True,
                stop=True,
            )
            bc1 = p_pv.tile([64, T], F32, tag="pv")
            nc.tensor.matmul(
                bc1[:],
                lhsT=ones64_t[:],
                rhs=rech_t[0:1, (2 * kc + 1) * T : (2 * kc + 2) * T],
                start=True,
                stop=True,
            )
            nc.vector.tensor_mul(ot_t[kc][0:64, :], pvacc_t[2 * kc][0:64, :], bc0[:])
            tmp1 = work.tile([64, T], BF16, tag="otmp", bufs=2)
            nc.vector.tensor_mul(tmp1[:], pvacc_t[2 * kc + 1][0:64, :], bc1[:])
            nc.sync.dma_start(ot_t[kc][64:128, :], tmp1[:])

        # ---- out projection -------------------------------------------------
        for eo in range(4):
            ps = p_pe.tile([128, 1024], F32, tag="pe")
            for cc in range(4):
                nc.tensor.matmul(
                    ps[:, 0:T],
                    lhsT=wo_t[cc][:, eo * 128 : (eo + 1) * 128],
                    rhs=ot_t[cc][:],
                    start=(cc == 0),
                    stop=(cc == 3),
                )
            osb = work.tile([128, T], F32, tag="osb", bufs=2)
            nc.vector.tensor_scalar_add(osb[:], ps[:, 0:T], bo_t[:, eo : eo + 1])
            nc.sync.dma_start(outT_d[eo], osb[:])

    nc.compile()
    return nc


_NC = None


def _get_nc():
    global _NC
    if _NC is None:
        _NC = _build_program()
    return _NC


def _prep_in_maps(x, context, key_padding_mask, Wq, Wkv, Wo, bo):
    wqT = (np.ascontiguousarray(Wq.T) * np.float32(D**-0.5)).astype(NPBF16)
    wkvT = np.ascontiguousarray(Wkv.T).astype(NPBF16)
    woT = np.ascontiguousarray(Wo.T).astype(NPBF16)
    bo_r = np.ascontiguousarray(bo.reshape(4, 128).T).astype(np.float32)
    in_maps = []
    for b in range(B):
        ctxT = np.ascontiguousarray(context[b].T).astype(NPBF16)
        xT = np.ascontiguousarray(x[b].T).astype(NPBF16)
        m01 = np.ascontiguousarray(
            (~key_padding_mask[b]).astype(np.float32).reshape(32, 128).T
        )
        in_maps.append(
            dict(ctxT=ctxT, xT=xT, m01=m01, wqT=wqT, wkvT=wkvT, woT=woT, bo_r=bo_r)
        )
    return in_maps


def _run(inputs, trace=False, **kw):
    nc = _get_nc()
    in_maps = _prep_in_maps(**inputs)
    res = bass_utils.run_bass_kernel_spmd(
        nc, in_maps, core_ids=list(range(NC_CORES)), trace=trace, **kw
    )
    out = np.stack(
        [res.results[b]["outT"].reshape(E, T).T for b in range(B)]
    ).astype(np.float32)
    return out, res


def kernel(**inputs):
    out, _ = _run(inputs, trace=False)
    return out


if __name__ == "__main__":
    rng = np.random.default_rng(0)
    ins = dict(
        x=rng.standard_normal((B, T, E), dtype=np.float32),
        context=rng.standard_normal((B, S, KV), dtype=np.float32),
        key_padding_mask=rng.integers(0, 2, (B, S)).astype(bool),
        Wq=(rng.standard_normal((512, E), dtype=np.float32) * 0.02),
        Wkv=(rng.standard_normal((1024, KV), dtype=np.float32) * 0.02),
        Wo=(rng.standard_normal((E, 512), dtype=np.float32) * 0.02),
        bo=np.zeros(E, dtype=np.float32),
    )
    out = kernel(**ins)
    print("out", out.shape, out.dtype, np.abs(out).mean())



# revision 11
# speedup vs baseline: 1.5087x; 1.5087x over previous
"""CrossAttention Trainium2 kernel.

Problem (hardcoded): B=8, T=256, S=4096, E=512, KV=768, H=8, D=64.
Sharding: data-parallel over B — one batch per NeuronCore (8 cores).

Key optimization vs v0: the key_padding_mask drops ~half the keys and
softmax attention is permutation-invariant over keys, so the host gathers
only the kept context rows per batch and pads to S_pad (multiple of 128,
same for all cores). Padded rows have zero context => K=0 => scores=0 =>
exp=1, but their V' rows and denominator ones-column are zero, so they
contribute nothing. This exactly halves KV-proj / scores / exp / PV work.

Per-core dataflow (one batch, layouts staged host-side, bf16 unless noted):
  ctxT  [768, S_pad]        context[b][kept].T, zero-padded
  xT    [512, 256]          x[b].T
  onesp [128, n_sc*8]       per-(s,head) denominator ones-column (0 for pads)
  wqT   [512, 512]          Wq.T * D^-0.5 (scale folded)
  wkvT  [768, 1024]         Wkv.T
  woT   [512, 512]          Wo.T
  bo_r  [128, 4] f32        bo.reshape(4,128).T
Device, per S piece (512 cols at a time):
  KT    = wkvT[:, :512].T @ ctxT   -> [512c, S_pad]  (c-major)
  V'    = ctxT.T @ wkvT[:, 512:]   -> [S_pad, 8h*65] (64 vals + ones col)
  scoresT[s,t]/head: KT_h slices as lhsT, QT_h as rhs (K=64; head pair in
          PE row groups 0:64 / 64:128 runs concurrently)
  expsT = Exp(scoresT) (no max subtraction: |scores| small by construction)
  PV   += V'_h-as-lhsT @ expsT -> [65, 256]; row 64 = softmax denominator
Tail: reciprocal of denominators, K=1 matmul broadcast, OT = PV * recip,
  outT = woT.T @ OT + bo -> [512e, 256t] -> host transposes back.

DMA: few big strided transfers spread across the sync/gpsimd/vector/scalar
queues (issue cost ~630ns/call), ctx piece 0 split across two queues so
compute starts ASAP.
"""

import sys

sys.path.insert(0, "/opt/trn_rl_repo")

import numpy as np
import ml_dtypes
from contextlib import ExitStack

import concourse.bass as bass
import concourse.bacc as bacc
import concourse.tile as tile
from concourse import mybir
from concourse import bass_utils

BF16 = mybir.dt.bfloat16
F32 = mybir.dt.float32
NPBF16 = ml_dtypes.bfloat16

B, T, S, E, KV, H, D = 8, 256, 4096, 512, 768, 8, 64
NC_CORES = 8


def _pieces(s_pad):
    out = []
    p0 = 0
    while p0 < s_pad:
        w = min(512, s_pad - p0)
        out.append((p0, w))
        p0 += w
    return out


def _build_program(s_pad, debug=False):
    n_sc = s_pad // 128
    pieces = _pieces(s_pad)
    nc = bacc.Bacc("TRN2", target_bir_lowering=False, debug=False)

    ctxT_d = nc.dram_tensor("ctxT", [KV, s_pad], BF16, kind="ExternalInput").ap()
    xT_d = nc.dram_tensor("xT", [E, T], BF16, kind="ExternalInput").ap()
    onesp_d = nc.dram_tensor("onesp", [128, n_sc * 8], BF16, kind="ExternalInput").ap()
    wqT_d = nc.dram_tensor("wqT", [E, 512], BF16, kind="ExternalInput").ap()
    wkvT_d = nc.dram_tensor("wkvT", [KV, 1024], BF16, kind="ExternalInput").ap()
    woT_d = nc.dram_tensor("woT", [512, E], BF16, kind="ExternalInput").ap()
    bo_d = nc.dram_tensor("bo_r", [128, 4], F32, kind="ExternalInput").ap()
    outT_d = nc.dram_tensor("outT", [4, 128, T], F32, kind="ExternalOutput").ap()
    if debug:
        dbg_qt = nc.dram_tensor("dbg_qt", [128, 4 * T], BF16, kind="ExternalOutput").ap()
        dbg_kt = nc.dram_tensor("dbg_kt", [128, 4 * s_pad], BF16, kind="ExternalOutput").ap()
        dbg_vp = nc.dram_tensor("dbg_vp", [128, n_sc * 8 * 65], BF16, kind="ExternalOutput").ap()
        dbg_den = nc.dram_tensor("dbg_den", [8, T], F32, kind="ExternalOutput").ap()
        dbg_pv = nc.dram_tensor("dbg_pv", [8, 65, T], F32, kind="ExternalOutput").ap()
        dbg_ot = nc.dram_tensor("dbg_ot", [4, 128, T], BF16, kind="ExternalOutput").ap()

    with tile.TileContext(nc) as tc, ExitStack() as ctx:
        const = ctx.enter_context(tc.tile_pool(name="const", bufs=1))
        work = ctx.enter_context(tc.tile_pool(name="work", bufs=2))
        p_pe = ctx.enter_context(tc.tile_pool(name="p_pe", bufs=3, space="PSUM"))
        p_pv = ctx.enter_context(tc.tile_pool(name="p_pv", bufs=2, space="PSUM"))

        # ---- static SBUF tensors -------------------------------------------
        ctx_t = const.tile([128, 6 * s_pad], BF16, tag="ctx")
        kt_t = const.tile([128, 4 * s_pad], BF16, tag="kt")
        vp_t = const.tile([128, n_sc * 8 * 65], BF16, tag="vp")
        qt_t = const.tile([128, 4 * T], BF16, tag="qt")
        ot_t = [const.tile([128, T], BF16, tag=f"ot{cc}", name=f"ot{cc}") for cc in range(4)]
        wq_t = const.tile([128, 4 * 512], BF16, tag="wq")
        wkv_t = const.tile([128, 6 * 1024], BF16, tag="wkv")
        wo_t = const.tile([128, 4 * 512], BF16, tag="wo")
        x_t = const.tile([128, 4 * T], BF16, tag="x")
        pvacc_t = [const.tile([65, T], F32, tag=f"pvacc{h}", name=f"pvacc{h}") for h in range(8)]
        den8_t = const.tile([8, T], F32, tag="den8")
        rec8_t = const.tile([8, T], F32, tag="rec8")
        rech_t = const.tile([1, 8 * T], F32, tag="rech")
        bo_t = const.tile([128, 4], F32, tag="bo")
        ones64_t = const.tile([1, 64], F32, tag="ones64")
        osb_t = const.tile([128, 4 * T], F32, tag="osb")
        m01h_t = const.tile([128, n_sc * 8], BF16, tag="m01h")

        vp_v = vp_t[:].rearrange("p (sc h e) -> p sc h e", h=8, e=65)

        # ---- loads ----------------------------------------------------------
        nc.vector.memset(ones64_t[:], 1.0)
        # Q-proj deps first on the scalar queue (scalar is idle until exp).
        nc.scalar.dma_start(
            x_t[:].rearrange("p (c t) -> p c t", t=T),
            xT_d.rearrange("(c p) t -> p c t", p=128),
        )
        nc.scalar.dma_start(
            wq_t[:].rearrange("p (c m) -> p c m", m=512),
            wqT_d.rearrange("(c p) m -> p c m", p=128),
        )
        # KV weights on gpsimd; ctx piece 0 on sync.
        nc.gpsimd.dma_start(
            wkv_t[:].rearrange("p (c m) -> p c m", m=1024),
            wkvT_d.rearrange("(c p) m -> p c m", p=128),
        )
        ctx_sb = ctx_t[:].rearrange("p (c s) -> p c s", s=s_pad)
        ctx_dr = ctxT_d.rearrange("(c p) s -> p c s", p=128)
        for i, (p0, w) in enumerate(pieces):
            eng = nc.sync if i % 2 == 0 else nc.gpsimd
            eng.dma_start(ctx_sb[:, :, p0 : p0 + w], ctx_dr[:, :, p0 : p0 + w])
        # denominator ones-column pattern (contiguous load; spread per piece)
        nc.scalar.dma_start(m01h_t[:], onesp_d)
        nc.sync.dma_start(
            wo_t[:].rearrange("p (c m) -> p c m", m=512),
            woT_d.rearrange("(c p) m -> p c m", p=128),
        )
        nc.sync.dma_start(bo_t[:], bo_d)

        # ---- Q projection ---------------------------------------------------
        ps_q = p_pe.tile([128, 1024], F32, tag="pe", name="pe")
        for qc in range(4):
            for ec in range(4):
                nc.tensor.matmul(
                    ps_q[:, qc * T : qc * T + T],
                    lhsT=wq_t[:, ec * 512 + qc * 128 : ec * 512 + (qc + 1) * 128],
                    rhs=x_t[:, ec * T : (ec + 1) * T],
                    start=(ec == 0),
                    stop=(ec == 3),
                )
        nc.vector.tensor_copy(qt_t[:], ps_q[:])

        # ---- main loop over S pieces ---------------------------------------
        for pi, (p0, w) in enumerate(pieces):
            scs = [p0 // 128 + j for j in range(w // 128)]
            # K projection: kc pairs share one psum tile
            for half in range(2):
                ps = p_pe.tile([128, 1024], F32, tag="pe", name="pe")
                for kci in range(2):
                    kc = 2 * half + kci
                    for c in range(6):
                        nc.tensor.matmul(
                            ps[:, kci * w : (kci + 1) * w],
                            lhsT=wkv_t[:, c * 1024 + kc * 128 : c * 1024 + (kc + 1) * 128],
                            rhs=ctx_t[:, c * s_pad + p0 : c * s_pad + p0 + w],
                            start=(c == 0),
                            stop=(c == 5),
                        )
                kt_v = kt_t[:].rearrange("p (kc s) -> p kc s", s=s_pad)
                nc.vector.tensor_copy(
                    kt_v[:, 2 * half : 2 * half + 2, p0 : p0 + w],
                    ps[:, 0 : 2 * w].rearrange("p (kc s) -> p kc s", s=w),
                )
            # V' projection: sc pairs share one psum tile
            for g in range(0, len(scs), 2):
                pair = scs[g : g + 2]
                ps = p_pe.tile([128, 1024], F32, tag="pe", name="pe")
                for j, sc in enumerate(pair):
                    for c in range(6):
                        nc.tensor.matmul(
                            ps[:, j * 512 : (j + 1) * 512],
                            lhsT=ctx_t[:, c * s_pad + sc * 128 : c * s_pad + (sc + 1) * 128],
                            rhs=wkv_t[:, c * 1024 + 512 : c * 1024 + 1024],
                            start=(c == 0),
                            stop=(c == 5),
                        )
                nc.vector.tensor_copy(
                    vp_v[:, pair[0] : pair[0] + len(pair), :, 0:64],
                    ps[:, 0 : len(pair) * 512].rearrange(
                        "p (sc h d) -> p sc h d", h=8, d=64
                    ),
                )
            nc.vector.tensor_copy(
                vp_v[:, scs[0] : scs[0] + len(scs), :, 64:65],
                m01h_t[:, scs[0] * 8 : (scs[0] + len(scs)) * 8].rearrange(
                    "p (sc h o) -> p sc h o", h=8, o=1
                ),
            )
            # scores + exp + PV, per head pair
            for kc in range(4):
                pe0 = p_pe.tile([128, 1024], F32, tag="pe", name="pe")
                pe1 = p_pe.tile([128, 1024], F32, tag="pe", name="pe")
                for j, sc in enumerate(scs):
                    nc.tensor.matmul(
                        pe0[:, j * T : (j + 1) * T],
                        lhsT=kt_t[0:64, kc * s_pad + sc * 128 : kc * s_pad + (sc + 1) * 128],
                        rhs=qt_t[0:64, kc * T : (kc + 1) * T],
                        start=True,
                        stop=True,
                    )
                    nc.tensor.matmul(
                        pe1[:, j * T : (j + 1) * T],
                        lhsT=kt_t[64:128, kc * s_pad + sc * 128 : kc * s_pad + (sc + 1) * 128],
                        rhs=qt_t[64:128, kc * T : (kc + 1) * T],
                        start=True,
                        stop=True,
                    )
                e0 = work.tile([128, 1024], BF16, tag="exp", bufs=6, name="exp")
                nc.scalar.activation(
                    e0[:, 0 : w * 2], pe0[:, 0 : w * 2], mybir.ActivationFunctionType.Exp
                )
                e1 = work.tile([128, 1024], BF16, tag="exp", bufs=6, name="exp")
                nc.scalar.activation(
                    e1[:, 0 : w * 2], pe1[:, 0 : w * 2], mybir.ActivationFunctionType.Exp
                )
                pv0 = p_pv.tile([65, T], F32, tag="pv", name="pv")
                pv1 = p_pv.tile([65, T], F32, tag="pv", name="pv")
                for j, sc in enumerate(scs):
                    o0 = (sc * 8 + 2 * kc) * 65
                    o1 = (sc * 8 + 2 * kc + 1) * 65
                    nc.tensor.matmul(
                        pv0[:],
                        lhsT=vp_t[:, o0 : o0 + 65],
                        rhs=e0[:, j * T : (j + 1) * T],
                        start=(j == 0),
                        stop=(j == len(scs) - 1),
                    )
                    nc.tensor.matmul(
                        pv1[:],
                        lhsT=vp_t[:, o1 : o1 + 65],
                        rhs=e1[:, j * T : (j + 1) * T],
                        start=(j == 0),
                        stop=(j == len(scs) - 1),
                    )
                if pi == 0:
                    nc.vector.tensor_copy(pvacc_t[2 * kc][:], pv0[:])
                    nc.vector.tensor_copy(pvacc_t[2 * kc + 1][:], pv1[:])
                else:
                    nc.vector.tensor_add(pvacc_t[2 * kc][:], pvacc_t[2 * kc][:], pv0[:])
                    nc.vector.tensor_add(
                        pvacc_t[2 * kc + 1][:], pvacc_t[2 * kc + 1][:], pv1[:]
                    )
                if pi == len(pieces) - 1:
                    nc.sync.dma_start(
                        den8_t[2 * kc : 2 * kc + 1, :], pvacc_t[2 * kc][64:65, :]
                    )
                    nc.gpsimd.dma_start(
                        den8_t[2 * kc + 1 : 2 * kc + 2, :],
                        pvacc_t[2 * kc + 1][64:65, :],
                    )

        # ---- deferred softmax normalization ---------------------------------
        nc.vector.reciprocal(rec8_t[:], den8_t[:])
        nc.sync.dma_start(
            rech_t[0:1, :].rearrange("p (h t) -> p h t", t=T), rec8_t[:, :]
        )
        for kc in range(4):
            bc0 = p_pv.tile([64, T], F32, tag="pv", name="pv")
            nc.tensor.matmul(
                bc0[:],
                lhsT=ones64_t[:],
                rhs=rech_t[0:1, (2 * kc) * T : (2 * kc + 1) * T],
                start=True,
                stop=True,
            )
            bc1 = p_pv.tile([64, T], F32, tag="pv", name="pv")
            nc.tensor.matmul(
                bc1[:],
                lhsT=ones64_t[:],
                rhs=rech_t[0:1, (2 * kc + 1) * T : (2 * kc + 2) * T],
                start=True,
                stop=True,
            )
            nc.vector.tensor_mul(ot_t[kc][0:64, :], pvacc_t[2 * kc][0:64, :], bc0[:])
            tmp1 = work.tile([64, T], BF16, tag="otmp", bufs=2, name="otmp")
            nc.vector.tensor_mul(tmp1[:], pvacc_t[2 * kc + 1][0:64, :], bc1[:])
            eng = nc.sync if kc % 2 == 0 else nc.gpsimd
            eng.dma_start(ot_t[kc][64:128, :], tmp1[:])

        # ---- out projection -------------------------------------------------
        for eo in range(4):
            ps = p_pe.tile([128, 1024], F32, tag="pe", name="pe")
            for cc in range(4):
                nc.tensor.matmul(
                    ps[:, 0:T],
                    lhsT=wo_t[:, cc * 512 + eo * 128 : cc * 512 + (eo + 1) * 128],
                    rhs=ot_t[cc][:],
                    start=(cc == 0),
                    stop=(cc == 3),
                )
            nc.vector.tensor_scalar_add(
                osb_t[:, eo * T : (eo + 1) * T], ps[:, 0:T], bo_t[:, eo : eo + 1]
            )
            eng = nc.sync if eo % 2 == 0 else nc.gpsimd
            eng.dma_start(outT_d[eo], osb_t[:, eo * T : (eo + 1) * T])
        if debug:
            nc.gpsimd.dma_start(dbg_qt, qt_t[:])
            nc.gpsimd.dma_start(dbg_kt, kt_t[:])
            nc.gpsimd.dma_start(dbg_vp, vp_t[:])
            nc.gpsimd.dma_start(dbg_den, den8_t[:])
            for h in range(8):
                nc.gpsimd.dma_start(dbg_pv[h], pvacc_t[h][:])
            for cc in range(4):
                nc.gpsimd.dma_start(dbg_ot[cc], ot_t[cc][:])

    nc.compile()
    return nc


_NC_CACHE = {}


def _get_nc(s_pad, debug=False):
    key = (s_pad, debug)
    if key not in _NC_CACHE:
        _NC_CACHE[key] = _build_program(s_pad, debug)
    return _NC_CACHE[key]


def _prep_in_maps(x, context, key_padding_mask, Wq, Wkv, Wo, bo):
    keep = ~np.asarray(key_padding_mask)
    kept = keep.sum(axis=1)
    s_pad = max(128, -(-int(kept.max()) // 128) * 128)
    n_sc = s_pad // 128

    wqT = (np.ascontiguousarray(Wq.T) * np.float32(D**-0.5)).astype(NPBF16)
    wkvT = np.ascontiguousarray(Wkv.T).astype(NPBF16)
    woT = np.ascontiguousarray(Wo.T).astype(NPBF16)
    bo_r = np.ascontiguousarray(bo.reshape(4, 128).T).astype(np.float32)
    in_maps = []
    for b in range(B):
        k = int(kept[b])
        ctxT = np.zeros((KV, s_pad), dtype=NPBF16)
        ctxT[:, :k] = context[b][keep[b]].T.astype(NPBF16)
        xT = np.ascontiguousarray(x[b].T).astype(NPBF16)
        # ones-column pattern: onesp[p, sc*8 + h] = 1 if sc*128+p < k
        live = (np.arange(s_pad) < k).astype(NPBF16).reshape(n_sc, 128).T
        onesp = np.ascontiguousarray(np.repeat(live, 8, axis=1))
        in_maps.append(
            dict(ctxT=ctxT, xT=xT, onesp=onesp, wqT=wqT, wkvT=wkvT, woT=woT, bo_r=bo_r)
        )
    return s_pad, in_maps


def _run(inputs, trace=False, debug=False, **kw):
    s_pad, in_maps = _prep_in_maps(**inputs)
    nc = _get_nc(s_pad, debug)
    res = bass_utils.run_bass_kernel_spmd(
        nc, in_maps, core_ids=list(range(NC_CORES)), trace=trace, **kw
    )
    out = np.stack(
        [res.results[b]["outT"].reshape(E, T).T for b in range(B)]
    ).astype(np.float32)
    return out, res


def kernel(**inputs):
    out, _ = _run(inputs, trace=False)
    return out


if __name__ == "__main__":
    rng = np.random.default_rng(0)
    ins = dict(
        x=rng.standard_normal((B, T, E), dtype=np.float32),
        context=rng.standard_normal((B, S, KV), dtype=np.float32),
        key_padding_mask=rng.integers(0, 2, (B, S)).astype(bool),
        Wq=(rng.standard_normal((512, E), dtype=np.float32) * 0.02),
        Wkv=(rng.standard_normal((1024, KV), dtype=np.float32) * 0.02),
        Wo=(rng.standard_normal((E, 512), dtype=np.float32) * 0.02),
        bo=np.zeros(E, dtype=np.float32),
    )
    out = kernel(**ins)
    print("out", out.shape, out.dtype, np.abs(out).mean())
